# revision 6
# baseline (speedup 1.0000x reference)
"""Segment-sum (scatter-add) kernel for Trainium2, SPMD over 8 NeuronCores.

Problem: out[n, :] = sum over edges e with X_node[e] == n of H[e, :]
  H [E=800000, 64] f32, X_node [E] int64, node_num N=50000 -> out [N, 64] f32.

Strategy
--------
Host-side sharding: edges are bucketed by destination node (each core owns a
contiguous node range chosen so per-core edge counts are ~equal).  Within a
core, nodes are greedily packed into "windows" of <= WN=32 consecutive nodes
whose edges fit in B blocks of 128 edges; every window is padded to exactly
B*128 edge slots so all 8 cores run one identical SPMD program.

Per 128-edge block the host precomputes a one-hot fp8 mask [128 edges x WN
nodes] (mask[e, j] = 1 iff X[e] == window_base + j) and splits H rows into a
2-way fp8(e4m3) cascade hi + mid == H to ~2^-10 relative (mid = fp8(H - hi),
unscaled - e4m3 subnormals carry the small residuals).  Pairs of blocks are
packed into 256-edge "super-blocks" in the DoubleRow interleave layout: per
partition [hi|mid(e0) | hi|mid(e1) | mask(e0) | mask(e1)], one fp8 tensor
(160 B/edge total - the kernel is HBM-bandwidth bound, so bytes are king).

Device kernel per core:
  PE:  psum[WN, 128] += mask.T @ [hi|mid] as ONE DoubleRow fp8 matmul
       per super-block; SB-super accumulation group per window; G_PS=16
       windows per 4-bank PSUM tile, 2 tiles ping-pong (all 8 banks).
  DVE: out = psum_hi + psum_mid -> [WN, D] f32 (one add per group)
  DMA: sync ring streams packed chunks (sizes ramp up at the start so the
       first matmul starts early, and ramp down at the end so the drain
       tail after the last load is short); gpsimd SWDGE ring does stores
       so they never queue ahead of loads.
Host gathers window rows into out[n0:n1, :] (pure layout, no arithmetic).
"""

import os

import numpy as np
import ml_dtypes

FP8 = np.dtype(ml_dtypes.float8_e4m3)

N_CORES = 8
P = 128
D = 64
WN = 32    # nodes per window (mask width)
G_PS = 16  # windows per PSUM tile / fold batch
G_TL = 4   # windows per fold batch for the tail groups
N_TL = 8   # number of tail windows folded in small batches
CH = 32    # steady-state super-blocks (256 edges each) per DMA chunk


def _chunk_plan(S):
    """Chunk sizes (in super-blocks): ramp up 4,4,8,16 so the first matmul
    only waits for a small chunk, steady CH, then ramp down 16,8,4 so the
    compute tail after the final load is short."""
    up = [4, 4, 8, 16]
    down = [16, 8, 4]
    if S <= sum(up) + sum(down):
        sizes = []
        t = 0
        while t < S:
            s = min(8, S - t)
            sizes.append(s)
            t += s
        return sizes
    mid = S - sum(up) - sum(down)
    sizes = list(up)
    while mid > 0:
        s = min(CH, mid)
        sizes.append(s)
        mid -= s
    sizes.extend(down)
    return sizes


def _group_plan(W):
    """Fold-group sizes (in windows): G_PS steady, but the last N_TL windows
    fold in batches of G_TL so the post-last-load tail is short."""
    tail = min(W, N_TL)
    head = W - tail
    gs = []
    w = 0
    while w < head:
        g = min(G_PS, head - w)
        gs.append(g)
        w += g
    while w < W:
        g = min(G_TL, W - w)
        gs.append(g)
        w += g
    return gs


# ----------------------------------------------------------------- planning
def _pack_windows(counts, n0, n1, B):
    """Greedily pack nodes [n0, n1) into windows of <=WN nodes whose total
    edge count fits in B*128 slots.  Returns list of (node_start, n_nodes)."""
    cap = B * P
    wins = []
    ws = n0
    acc = 0
    nn = 0
    for n in range(n0, n1):
        c = int(counts[n])
        if nn == WN or (acc + c > cap and nn > 0):
            wins.append((ws, nn))
            ws, acc, nn = n, 0, 0
        if c > cap:
            return None  # single node exceeds capacity; need bigger B
        acc += c
        nn += 1
    if nn > 0:
        wins.append((ws, nn))
    return wins


def _plan(X, N):
    """Choose core node ranges, B (blocks/window) and W (windows/core)."""
    E = X.shape[0]
    order = np.argsort(X, kind="stable")
    Xs = X[order]
    counts = np.bincount(X, minlength=N)
    cum = np.zeros(N + 1, dtype=np.int64)
    np.cumsum(counts, out=cum[1:])

    nb = [0]
    for c in range(1, N_CORES):
        nb.append(int(np.searchsorted(cum, round(E * c / N_CORES), side="left")))
    nb.append(N)

    b_lo = max(2, -(-int(counts.max()) // P))
    b_lo += b_lo % 2  # DoubleRow pairs blocks: B must be even
    best = None
    for B in range(b_lo, b_lo + 24, 2):
        wins_all = []
        ok = True
        for c in range(N_CORES):
            wins = _pack_windows(counts, nb[c], nb[c + 1], B)
            if wins is None:
                ok = False
                break
            wins_all.append(wins)
        if not ok:
            continue
        W = max(len(w) for w in wins_all)
        cost = W * B  # proportional to padded edges (dominant DMA)
        if best is None or cost < best[0]:
            best = (cost, B, W, wins_all)
    assert best is not None, "window packing failed"
    _, B, W, wins_all = best
    return order, Xs, cum, nb, B, W, wins_all


def _build_core_inputs(H32, order, Xs, cum, wins, B, W):
    """Build the padded, reordered device inputs for one core."""
    T = W * B
    idx = np.full(T * P, -1, dtype=np.int64)
    off = np.full(T * P, 255, dtype=np.int64)  # >= WN: all-zero mask row
    for w, (ns, nn) in enumerate(wins):
        e0 = int(cum[ns])
        e1 = int(cum[ns + nn])
        ec = e1 - e0
        s = w * B * P
        idx[s : s + ec] = order[e0:e1]
        off[s : s + ec] = Xs[e0:e1] - ns

    Hg = H32[np.maximum(idx, 0)]
    Hg[idx < 0] = 0.0
    # 2-way fp8 cascade: hi + mid == H to ~2^-10 relative.  mid is stored
    # UNSCALED: e4m3 subnormals (granularity 2^-9) carry small residuals,
    # so both terms accumulate in PSUM with no scaling fold needed.
    hi = Hg.astype(FP8)
    mid = (Hg - hi.astype(np.float32)).astype(FP8)
    msk = (off[:, None] == np.arange(WN)[None, :]).astype(FP8)  # [T*P, WN]
    # DoubleRow super-block layout (2 blocks interleave on the k axis):
    # per partition row: [hi|mid(e0) | hi|mid(e1) | mask(e0) | mask(e1)]
    S = T // 2
    H2 = np.concatenate([hi, mid], axis=1).reshape(S, 2, P, 2 * D)
    Mr = msk.reshape(S, 2, P, WN)
    pk = np.concatenate(
        [
            H2.transpose(0, 2, 1, 3).reshape(S, P, 4 * D),
            Mr.transpose(0, 2, 1, 3).reshape(S, P, 2 * WN),
        ],
        axis=2,
    )  # [S, P, 4D+2WN]
    pkt = np.ascontiguousarray(
        pk.transpose(1, 0, 2).reshape(P, S * (4 * D + 2 * WN))
    )
    return pkt


# ------------------------------------------------------------- device kernel
def _build_program(T, W, B):
    import concourse.bacc as bacc
    import concourse.tile as tile
    import concourse.mybir as mybir

    nc = bacc.Bacc("TRN2", target_bir_lowering=False, debug=False)
    fp8 = mybir.dt.float8e4
    f32 = mybir.dt.float32

    PKW = 4 * D + 2 * WN  # packed fp8 super-row: [hi|mid(e0)|hi|mid(e1)|m0|m1]
    SB = B // 2           # super-blocks per window
    S = T // 2
    PSW = 2 * D           # psum per-window region: [WN, 128] f32 = 512 B
    with tile.TileContext(nc) as tc:
        with tc.tile_pool(name="dram", bufs=1, space="DRAM") as dram:
            pkt = dram.tile([P, S * PKW], fp8, kind="ExternalInput")
            # [WN, W*D] layout: each fold's store is g*D f32 contiguous
            # per partition
            odev = dram.tile([WN, W * D], f32, kind="ExternalOutput")

            with tc.tile_pool(name="hbuf", bufs=4) as hpool, \
                 tc.tile_pool(name="psum", bufs=2, space="PSUM") as pspool, \
                 tc.tile_pool(name="outb", bufs=4) as opool:

                chunk_starts = {}
                t_acc = 0
                for s in _chunk_plan(S):
                    chunk_starts[t_acc] = s
                    t_acc += s

                groups = _group_plan(W)

                pk = None
                t0 = 0
                ps = None
                gi = 0      # group index
                g0 = 0      # first window of current group
                for w in range(W):
                    g = w - g0
                    ng = groups[gi]
                    if g == 0:
                        ps = pspool.tile([WN, ng, PSW], f32)
                    for b in range(SB):
                        t = w * SB + b
                        if t in chunk_starts:
                            ch = chunk_starts[t]
                            t0 = t
                            pk = hpool.tile([P, CH, PKW], fp8, tag="h")
                            nc.sync.dma_start(
                                out=pk[:, :ch, :],
                                in_=pkt[:, t * PKW : (t + ch) * PKW].rearrange(
                                    "p (c d) -> p c d", c=ch
                                ),
                            )
                        rel = t - t0
                        nc.tensor.matmul(
                            out=ps[:, g, :],
                            lhsT=pk[:, rel, 4 * D : PKW].rearrange(
                                "p (k m) -> p k m", k=2
                            ),
                            rhs=pk[:, rel, 0 : 4 * D].rearrange(
                                "p (k n) -> p k n", k=2
                            ),
                            start=(b == 0),
                            stop=(b == SB - 1),
                            perf_mode=mybir.MatmulPerfMode.DoubleRow,
                        )
                    if g == ng - 1:
                        # fold: out = psum_hi + psum_mid (no scaling).  The
                        # DVE can't read two PSUM operands in one op, so ACT
                        # stages the mid half through SBUF (also splits the
                        # PSUM-release work across two engines).
                        tm = opool.tile([WN, ng, D], f32, tag="t")
                        nc.scalar.copy(out=tm[:, :, :], in_=ps[:, :, D : 2 * D])
                        ot = opool.tile([WN, ng, D], f32, tag="o")
                        nc.vector.tensor_tensor(
                            out=ot[:, :, :],
                            in0=ps[:, :, 0:D],
                            in1=tm[:, :, :],
                            op=mybir.AluOpType.add,
                        )
                        # scalar-engine HW ring: separate queue from the
                        # sync ring so stores never block chunk loads, and
                        # it drains fast (unlike gpsimd SWDGE at ~22 GB/s)
                        nc.scalar.dma_start(
                            out=odev[:, g0 * D : (w + 1) * D].rearrange(
                                "n (g f) -> n g f", g=ng
                            ),
                            in_=ot[:, :, :],
                        )
                        gi += 1
                        g0 = w + 1
    nc.compile()
    return nc, pkt, odev


# --------------------------------------------------------------------- main
def kernel(H, X_node, node_num):
    from concourse import bass_utils

    H32 = np.asarray(H, dtype=np.float32)
    X = np.asarray(X_node).astype(np.int64)
    N = int(node_num)
    E = X.shape[0]
    assert H32.shape == (E, D)

    order, Xs, cum, nb, B, W, wins_all = _plan(X, N)
    T = W * B

    nc, pkt, odev = _build_program(T, W, B)
    in_maps = []
    for c in range(N_CORES):
        pkt_np = _build_core_inputs(H32, order, Xs, cum, wins_all[c], B, W)
        in_maps.append({pkt.name: pkt_np})

    trace = bool(int(os.environ.get("SEGSUM_TRACE", "0")))
    res = bass_utils.run_bass_kernel_spmd(
        nc, in_maps, core_ids=list(range(N_CORES)), trace=trace
    )
    if trace:
        kernel.last_exec_time_ns = res.exec_time_ns
        kernel.last_mean_exec_time_ns = res.mean_exec_time_ns
        kernel.last_trace = (
            res.instructions_and_trace[1] if res.instructions_and_trace else None
        )

    out = np.zeros((N, D), dtype=np.float32)
    for c in range(N_CORES):
        ot = res.results[c][odev.name].reshape(WN, W, D)  # [node_off, w, D]
        for w, (ns, nn) in enumerate(wins_all[c]):
            out[ns : ns + nn, :] = ot[:nn, w, :]
    return out


# revision 7
# speedup vs baseline: 1.0393x; 1.0393x over previous
"""Segment-sum (scatter-add) kernel for Trainium2, SPMD over 8 NeuronCores.

Problem: out[n, :] = sum over edges e with X_node[e] == n of H[e, :]
  H [E=800000, 64] f32, X_node [E] int64, node_num N=50000 -> out [N, 64] f32.

Strategy
--------
Host-side sharding: edges are bucketed by destination node (each core owns a
contiguous node range chosen so per-core edge counts are ~equal).  Within a
core, nodes are greedily packed into "windows" of <= WN=32 consecutive nodes
whose edges fit in B blocks of 128 edges; every window is padded to exactly
B*128 edge slots so all 8 cores run one identical SPMD program.

Per 128-edge block the host precomputes a one-hot fp8 mask [128 edges x WN
nodes] (mask[e, j] = 1 iff X[e] == window_base + j) and splits H rows into a
2-way fp8(e4m3) cascade hi + mid == H to ~2^-10 relative (mid = fp8(H - hi),
unscaled - e4m3 subnormals carry the small residuals).  Pairs of blocks are
packed into 256-edge "super-blocks" in the DoubleRow interleave layout: per
partition [hi|mid(e0) | hi|mid(e1) | mask(e0) | mask(e1)], one fp8 tensor
(160 B/edge total - the kernel is HBM-bandwidth bound, so bytes are king).

Device kernel per core:
  PE:  psum[WN, 128] += mask.T @ [hi|mid] as ONE DoubleRow fp8 matmul
       per super-block; SB-super accumulation group per window; G_PS=16
       windows per 4-bank PSUM tile, 2 tiles ping-pong (all 8 banks).
  DVE: out = psum_hi + psum_mid -> [WN, D] f32 (one add per group)
  DMA: sync ring streams packed chunks (sizes ramp up at the start so the
       first matmul starts early, and ramp down at the end so the drain
       tail after the last load is short); gpsimd SWDGE ring does stores
       so they never queue ahead of loads.
Host gathers window rows into out[n0:n1, :] (pure layout, no arithmetic).
"""

import os

import numpy as np
import ml_dtypes

FP8 = np.dtype(ml_dtypes.float8_e4m3)

N_CORES = 8
P = 128
D = 64
WN = 32    # nodes per window (mask width)
G_PS = 16  # windows per PSUM tile / fold batch
G_TL = 4   # windows per fold batch for the tail groups
N_TL = 8   # number of tail windows folded in small batches
CH = 32    # steady-state super-blocks (256 edges each) per DMA chunk


def _chunk_plan(S):
    """Chunk sizes (in super-blocks): ramp up 4,4,8,16 so the first matmul
    only waits for a small chunk, steady CH, then ramp down 16,8,4 so the
    compute tail after the final load is short."""
    up = [4, 4, 8, 16]
    down = [16, 8, 4]
    if S <= sum(up) + sum(down):
        sizes = []
        t = 0
        while t < S:
            s = min(8, S - t)
            sizes.append(s)
            t += s
        return sizes
    mid = S - sum(up) - sum(down)
    sizes = list(up)
    while mid > 0:
        s = min(CH, mid)
        sizes.append(s)
        mid -= s
    sizes.extend(down)
    return sizes


def _group_plan(W):
    """Fold-group sizes (in windows): G_PS steady, but the last N_TL windows
    fold in batches of G_TL so the post-last-load tail is short."""
    tail = min(W, N_TL)
    head = W - tail
    gs = []
    w = 0
    while w < head:
        g = min(G_PS, head - w)
        gs.append(g)
        w += g
    while w < W:
        g = min(G_TL, W - w)
        gs.append(g)
        w += g
    return gs


# ----------------------------------------------------------------- planning
def _pack_windows(counts, n0, n1, B):
    """Greedily pack nodes [n0, n1) into windows of <=WN nodes whose total
    edge count fits in B*128 slots.  Returns list of (node_start, n_nodes)."""
    cap = B * P
    wins = []
    ws = n0
    acc = 0
    nn = 0
    for n in range(n0, n1):
        c = int(counts[n])
        if nn == WN or (acc + c > cap and nn > 0):
            wins.append((ws, nn))
            ws, acc, nn = n, 0, 0
        if c > cap:
            return None  # single node exceeds capacity; need bigger B
        acc += c
        nn += 1
    if nn > 0:
        wins.append((ws, nn))
    return wins


def _plan(X, N):
    """Choose core node ranges, B (blocks/window) and W (windows/core)."""
    E = X.shape[0]
    order = np.argsort(X, kind="stable")
    Xs = X[order]
    counts = np.bincount(X, minlength=N)
    cum = np.zeros(N + 1, dtype=np.int64)
    np.cumsum(counts, out=cum[1:])

    nb = [0]
    for c in range(1, N_CORES):
        nb.append(int(np.searchsorted(cum, round(E * c / N_CORES), side="left")))
    nb.append(N)

    b_lo = max(2, -(-int(counts.max()) // P))
    b_lo += b_lo % 2  # DoubleRow pairs blocks: B must be even
    best = None
    for B in range(b_lo, b_lo + 24, 2):
        wins_all = []
        ok = True
        for c in range(N_CORES):
            wins = _pack_windows(counts, nb[c], nb[c + 1], B)
            if wins is None:
                ok = False
                break
            wins_all.append(wins)
        if not ok:
            continue
        W = max(len(w) for w in wins_all)
        cost = W * B  # proportional to padded edges (dominant DMA)
        if best is None or cost < best[0]:
            best = (cost, B, W, wins_all)
    assert best is not None, "window packing failed"
    _, B, W, wins_all = best
    return order, Xs, cum, nb, B, W, wins_all


def _build_core_inputs(H32, order, Xs, cum, wins, B, W):
    """Build the padded, reordered device inputs for one core."""
    T = W * B
    idx = np.full(T * P, -1, dtype=np.int64)
    off = np.full(T * P, 255, dtype=np.int64)  # >= WN: all-zero mask row
    for w, (ns, nn) in enumerate(wins):
        e0 = int(cum[ns])
        e1 = int(cum[ns + nn])
        ec = e1 - e0
        s = w * B * P
        idx[s : s + ec] = order[e0:e1]
        off[s : s + ec] = Xs[e0:e1] - ns

    Hg = H32[np.maximum(idx, 0)]
    Hg[idx < 0] = 0.0
    # 2-way fp8 cascade: hi + mid == H to ~2^-10 relative.  mid is stored
    # UNSCALED: e4m3 subnormals (granularity 2^-9) carry small residuals,
    # so both terms accumulate in PSUM with no scaling fold needed.
    hi = Hg.astype(FP8)
    mid = (Hg - hi.astype(np.float32)).astype(FP8)
    msk = (off[:, None] == np.arange(WN)[None, :]).astype(FP8)  # [T*P, WN]
    # DoubleRow super-block layout (2 blocks interleave on the k axis):
    # per partition row: [hi|mid(e0) | hi|mid(e1) | mask(e0) | mask(e1)]
    S = T // 2
    H2 = np.concatenate([hi, mid], axis=1).reshape(S, 2, P, 2 * D)
    Mr = msk.reshape(S, 2, P, WN)
    pk = np.concatenate(
        [
            H2.transpose(0, 2, 1, 3).reshape(S, P, 4 * D),
            Mr.transpose(0, 2, 1, 3).reshape(S, P, 2 * WN),
        ],
        axis=2,
    )  # [S, P, 4D+2WN]
    pkt = np.ascontiguousarray(
        pk.transpose(1, 0, 2).reshape(P, S * (4 * D + 2 * WN))
    )
    return pkt


# ------------------------------------------------------------- device kernel
def _build_program(T, W, B):
    import concourse.bacc as bacc
    import concourse.tile as tile
    import concourse.mybir as mybir

    nc = bacc.Bacc("TRN2", target_bir_lowering=False, debug=False)
    fp8 = mybir.dt.float8e4
    f32 = mybir.dt.float32

    PKW = 4 * D + 2 * WN  # packed fp8 super-row: [hi|mid(e0)|hi|mid(e1)|m0|m1]
    SB = B // 2           # super-blocks per window
    S = T // 2
    PSW = 2 * D           # psum per-window region: [WN, 128] f32 = 512 B
    with tile.TileContext(nc) as tc:
        with tc.tile_pool(name="dram", bufs=1, space="DRAM") as dram:
            pkt = dram.tile([P, S * PKW], fp8, kind="ExternalInput")
            # [WN, W*D] layout: each fold's store is g*D f32 contiguous
            # per partition
            odev = dram.tile([WN, W * D], f32, kind="ExternalOutput")

            with tc.tile_pool(name="hbuf", bufs=4) as hpool, \
                 tc.tile_pool(name="psum", bufs=2, space="PSUM") as pspool, \
                 tc.tile_pool(name="outb", bufs=4) as opool:

                chunk_starts = {}
                t_acc = 0
                for s in _chunk_plan(S):
                    chunk_starts[t_acc] = s
                    t_acc += s

                groups = _group_plan(W)

                pk = None
                t0 = 0
                ps = None
                gi = 0      # group index
                g0 = 0      # first window of current group
                for w in range(W):
                    g = w - g0
                    ng = groups[gi]
                    if g == 0:
                        ps = pspool.tile([WN, ng, PSW], f32)
                    for b in range(SB):
                        t = w * SB + b
                        if t in chunk_starts:
                            ch = chunk_starts[t]
                            t0 = t
                            pk = hpool.tile([P, CH, PKW], fp8, tag="h")
                            nc.sync.dma_start(
                                out=pk[:, :ch, :],
                                in_=pkt[:, t * PKW : (t + ch) * PKW].rearrange(
                                    "p (c d) -> p c d", c=ch
                                ),
                            )
                        rel = t - t0
                        nc.tensor.matmul(
                            out=ps[:, g, :],
                            lhsT=pk[:, rel, 4 * D : PKW].rearrange(
                                "p (k m) -> p k m", k=2
                            ),
                            rhs=pk[:, rel, 0 : 4 * D].rearrange(
                                "p (k n) -> p k n", k=2
                            ),
                            start=(b == 0),
                            stop=(b == SB - 1),
                            perf_mode=mybir.MatmulPerfMode.DoubleRow,
                        )
                    if g == ng - 1:
                        # fold: out = psum_hi + psum_mid (no scaling).  The
                        # DVE can't read two PSUM operands in one op, so ACT
                        # stages the mid half through SBUF (also splits the
                        # PSUM-release work across two engines).
                        tm = opool.tile([WN, ng, D], f32, tag="t")
                        nc.scalar.copy(out=tm[:, :, :], in_=ps[:, :, D : 2 * D])
                        ot = opool.tile([WN, ng, D], f32, tag="o")
                        nc.vector.tensor_tensor(
                            out=ot[:, :, :],
                            in0=ps[:, :, 0:D],
                            in1=tm[:, :, :],
                            op=mybir.AluOpType.add,
                        )
                        # SWDGE ring: parallel to the HW-DGE load path, so
                        # stores never steal load bandwidth
                        nc.gpsimd.dma_start(
                            out=odev[:, g0 * D : (w + 1) * D].rearrange(
                                "n (g f) -> n g f", g=ng
                            ),
                            in_=ot[:, :, :],
                        )
                        gi += 1
                        g0 = w + 1
    nc.compile()
    return nc, pkt, odev


# --------------------------------------------------------------------- main
def kernel(H, X_node, node_num):
    from concourse import bass_utils

    H32 = np.asarray(H, dtype=np.float32)
    X = np.asarray(X_node).astype(np.int64)
    N = int(node_num)
    E = X.shape[0]
    assert H32.shape == (E, D)

    order, Xs, cum, nb, B, W, wins_all = _plan(X, N)
    T = W * B

    nc, pkt, odev = _build_program(T, W, B)
    in_maps = []
    for c in range(N_CORES):
        pkt_np = _build_core_inputs(H32, order, Xs, cum, wins_all[c], B, W)
        in_maps.append({pkt.name: pkt_np})

    trace = bool(int(os.environ.get("SEGSUM_TRACE", "0")))
    res = bass_utils.run_bass_kernel_spmd(
        nc, in_maps, core_ids=list(range(N_CORES)), trace=trace
    )
    if trace:
        kernel.last_exec_time_ns = res.exec_time_ns
        kernel.last_mean_exec_time_ns = res.mean_exec_time_ns
        kernel.last_trace = (
            res.instructions_and_trace[1] if res.instructions_and_trace else None
        )

    out = np.zeros((N, D), dtype=np.float32)
    for c in range(N_CORES):
        ot = res.results[c][odev.name].reshape(WN, W, D)  # [node_off, w, D]
        for w, (ns, nn) in enumerate(wins_all[c]):
            out[ns : ns + nn, :] = ot[:nn, w, :]
    return out


# revision 8
# speedup vs baseline: 1.1256x; 1.0831x over previous
"""Segment-sum (scatter-add) kernel for Trainium2, SPMD over 8 NeuronCores.

Problem: out[n, :] = sum over edges e with X_node[e] == n of H[e, :]
  H [E=800000, 64] f32, X_node [E] int64, node_num N=50000 -> out [N, 64] f32.

Strategy (canonical class masks - no per-edge mask bytes)
---------------------------------------------------------
Nodes are sorted by edge count and grouped into "windows" of nn same-class
nodes, where a class is a count value c (counts < 8 pad up to 8, leftovers
lift into the next class so every class's window count is divisible by 8).
A window has a fixed capacity cap(c) in {256, 512, 768, 1024} slots chosen
to minimize slack, nn = min(32, cap // c) nodes, and node j's edges occupy
slots [j*c, j*c + count_j) (rest zero).  The one-hot mask for a window is
therefore a CANONICAL stripe pattern depending only on the class - the ~23
class masks (~290 KB fp8) are DMA'd once and stay resident in SBUF, so the
edge stream carries only data: a 2-way fp8(e4m3) cascade hi + mid == H to
~2^-10 relative (mid = fp8(H - hi) unscaled, e4m3 subnormals carry small
residuals), 128 B/edge-slot at ~2.7% slot padding.

The global window sequence (class runs divisible by 8) is dealt round-robin
to the 8 cores, so every core runs ONE identical program with the same
hardcoded class schedule.  Pairs of 128-slot blocks pack into 256-slot
super-blocks in the DoubleRow interleave layout: per partition
[hi|mid(e0) | hi|mid(e1)], with the resident class mask as lhsT.

Device kernel per core:
  PE:  psum[32, 128] += stripe_mask.T @ [hi|mid] as ONE DoubleRow fp8
       matmul per super-block; SB_w-super accumulation group per window;
       G_PS=16 windows per 4-bank PSUM tile, 2 tiles ping-pong.
  ACT: tmp = psum_mid (PSUM->SBUF copy, batched over the group)
  DVE: out = psum_hi + tmp -> [32, D] f32
  DMA: sync ring streams packed chunks (ramped sizes both ends: quick
       start, short drain); gpsimd SWDGE ring does stores (parallel to the
       HW-DGE load path, so stores never steal load bandwidth).
Host gathers window rows into out (pure layout, no arithmetic).
"""

import os

import numpy as np
import ml_dtypes

FP8 = np.dtype(ml_dtypes.float8_e4m3)

N_CORES = 8
P = 128
D = 64
WN = 32    # mask width: max nodes per window
G_PS = 16  # windows per PSUM tile / fold batch
G_TL = 4   # windows per fold batch for the tail groups
N_TL = 8   # number of tail windows folded in small batches
CH = 32    # steady-state super-blocks (256 slots each) per DMA chunk
CMIN = 8   # minimum class: smaller counts pad up


def _chunk_plan(S):
    """Chunk sizes (in super-blocks): ramp up 4,4,8,16 so the first matmul
    only waits for a small chunk, steady CH, then ramp down 16,8,4 so the
    compute tail after the final load is short."""
    up = [4, 4, 8, 16]
    down = [16, 8, 4]
    if S <= sum(up) + sum(down):
        sizes = []
        t = 0
        while t < S:
            s = min(8, S - t)
            sizes.append(s)
            t += s
        return sizes
    mid = S - sum(up) - sum(down)
    sizes = list(up)
    while mid > 0:
        s = min(CH, mid)
        sizes.append(s)
        mid -= s
    sizes.extend(down)
    return sizes


def _group_plan(W):
    """Fold-group sizes (in windows): G_PS steady, but the last N_TL windows
    fold in batches of G_TL so the post-last-load tail is short."""
    tail = min(W, N_TL)
    head = W - tail
    gs = []
    w = 0
    while w < head:
        g = min(G_PS, head - w)
        gs.append(g)
        w += g
    while w < W:
        g = min(G_TL, W - w)
        gs.append(g)
        w += g
    return gs


# ----------------------------------------------------------------- planning
def _nn_cap(c):
    """Best (nn, cap) for class c: capacity multiple of 256, nn <= WN,
    minimizing slack fraction."""
    best = None
    for cap in (256, 512, 768, 1024):
        nn = min(WN, cap // c)
        if nn == 0:
            continue
        key = ((cap - nn * c) / cap, cap)
        if best is None or key < best[0]:
            best = (key, cap, nn)
    return best[2], best[1]


def _plan(X, N):
    """Global class schedule + per-core window node lists.

    Returns (sched, wins_core, cum, order) where
      sched: list of (c, nn, cap) per GLOBAL window index (identical class
             sequence for every core once dealt round-robin),
      wins_core[core]: list of node-id arrays (one per local window),
      cum/order: edge sort fanout (cum[n]..cum[n+1] slices order -> edge ids
             of node n).
    """
    order = np.argsort(X, kind="stable")
    counts = np.bincount(X, minlength=N)
    cum = np.zeros(N + 1, dtype=np.int64)
    np.cumsum(counts, out=cum[1:])

    cmax = int(counts.max())
    cmax = max(cmax, CMIN)
    # pool[c] = node ids assigned to class c (count <= c)
    pools = {c: [] for c in range(CMIN, cmax + 2)}
    csort = np.argsort(counts, kind="stable")  # ascending count
    for n in csort:
        pools[max(CMIN, int(counts[n]))].append(int(n))

    gw = []  # global windows: (c, nn, cap, node_list)
    for c in range(CMIN, cmax + 1):
        nn, cap = _nn_cap(c)
        k = len(pools[c])
        w8 = (k // (nn * 8)) * 8
        used = w8 * nn
        if k - used:
            pools[c + 1] = pools[c][used:] + pools[c + 1]  # lift leftovers
        for w in range(w8):
            gw.append((c, nn, cap, pools[c][w * nn : (w + 1) * nn]))
    # top-level leftovers: final class, padded to a multiple of 8 windows
    c = cmax + 1
    nn, cap = _nn_cap(c)
    k = len(pools[c])
    w8 = -(-max(k, 1) // nn)
    w8 = -(-w8 // 8) * 8
    for w in range(w8):
        gw.append((c, nn, cap, pools[c][w * nn : (w + 1) * nn]))

    sched = [(c, nn, cap) for c, nn, cap, _ in gw[:: N_CORES]]
    wins_core = [
        [nodes for _, _, _, nodes in gw[cr::N_CORES]] for cr in range(N_CORES)
    ]
    return sched, wins_core, cum, order


def _build_masks(sched):
    """Canonical stripe masks, one per distinct class, DoubleRow-interleaved.

    Returns (mask_blob [P, MW] fp8, class_off dict c -> column offset).
    Per class the layout is SB super-blocks x [P, 2, WN]."""
    classes = []
    seen = set()
    for c, nn, cap in sched:
        if c not in seen:
            seen.add(c)
            classes.append((c, nn, cap))
    cols = []
    class_off = {}
    off = 0
    for c, nn, cap in classes:
        slots = np.arange(cap)
        node = slots // c  # stripe: slot s -> node s//c
        msk = (node[:, None] == np.arange(WN)[None, :]) & (node[:, None] < nn)
        msk = msk.astype(FP8)  # [cap, WN]
        sb = cap // 256
        m = msk.reshape(sb, 2, P, WN).transpose(0, 2, 1, 3)  # [sb, P, 2, WN]
        cols.append(m.reshape(sb, P, 2 * WN))
        class_off[c] = off
        off += sb * 2 * WN
    blob = np.concatenate(cols, axis=0)  # [SBtot, P, 2*WN]
    blob = np.ascontiguousarray(blob.transpose(1, 0, 2).reshape(P, off))
    return blob, class_off


def _build_core_inputs(H32, cum, order, wins, sched):
    """Packed [hi|mid] DoubleRow edge stream for one core."""
    S = sum(cap // 256 for _, _, cap in sched)
    Hg = np.zeros((S * 2 * P, D), dtype=np.float32)
    s0 = 0  # slot cursor
    for (c, nn, cap), nodes in zip(sched, wins, strict=True):
        for j, n in enumerate(nodes):
            e0 = int(cum[n])
            e1 = int(cum[n + 1])
            base = s0 + j * c
            Hg[base : base + (e1 - e0)] = H32[order[e0:e1]]
        s0 += cap
    # 2-way fp8 cascade: hi + mid == H to ~2^-10 relative (mid unscaled,
    # e4m3 subnormals carry the small residuals; accumulates fold-free).
    hi = Hg.astype(FP8)
    mid = (Hg - hi.astype(np.float32)).astype(FP8)
    H2 = np.concatenate([hi, mid], axis=1)  # [S*2*P, 2D]
    pk = H2.reshape(S, 2, P, 2 * D).transpose(2, 0, 1, 3)  # [P, S, 2, 2D]
    return np.ascontiguousarray(pk.reshape(P, S * 4 * D))


# ------------------------------------------------------------- device kernel
def _build_program(sched, class_off, MW):
    import concourse.bacc as bacc
    import concourse.tile as tile
    import concourse.mybir as mybir

    nc = bacc.Bacc("TRN2", target_bir_lowering=False, debug=False)
    fp8 = mybir.dt.float8e4
    f32 = mybir.dt.float32

    PKW = 4 * D  # packed fp8 super-row: [hi|mid(e0) | hi|mid(e1)]
    W = len(sched)
    S = sum(cap // 256 for _, _, cap in sched)
    with tile.TileContext(nc) as tc:
        with tc.tile_pool(name="dram", bufs=1, space="DRAM") as dram:
            pkt = dram.tile([P, S * PKW], fp8, kind="ExternalInput")
            mskd = dram.tile([P, MW], fp8, kind="ExternalInput")
            odev = dram.tile([WN, W * D], f32, kind="ExternalOutput")

            with tc.tile_pool(name="mbuf", bufs=1) as mpool, \
                 tc.tile_pool(name="hbuf", bufs=6) as hpool, \
                 tc.tile_pool(name="psum", bufs=2, space="PSUM") as pspool, \
                 tc.tile_pool(name="outb", bufs=6) as opool:

                msk = mpool.tile([P, MW], fp8)
                nc.sync.dma_start(out=msk, in_=mskd)

                chunk_starts = {}
                t_acc = 0
                for s in _chunk_plan(S):
                    chunk_starts[t_acc] = s
                    t_acc += s

                groups = _group_plan(W)

                pk = None
                t0 = 0
                t = 0
                ps = None
                gi = 0      # group index
                g0 = 0      # first window of current group
                for w, (c, nn, cap) in enumerate(sched):
                    g = w - g0
                    ng = groups[gi]
                    if g == 0:
                        ps = pspool.tile([WN, ng, 2 * D], f32)
                    SB = cap // 256
                    moff = class_off[c]
                    for b in range(SB):
                        if t in chunk_starts:
                            ch = chunk_starts[t]
                            t0 = t
                            pk = hpool.tile([P, CH, PKW], fp8, tag="h")
                            nc.sync.dma_start(
                                out=pk[:, :ch, :],
                                in_=pkt[:, t * PKW : (t + ch) * PKW].rearrange(
                                    "p (c d) -> p c d", c=ch
                                ),
                            )
                        rel = t - t0
                        nc.tensor.matmul(
                            out=ps[:, g, :],
                            lhsT=msk[
                                :, moff + b * 2 * WN : moff + (b + 1) * 2 * WN
                            ].rearrange("p (k m) -> p k m", k=2),
                            rhs=pk[:, rel, :].rearrange("p (k n) -> p k n", k=2),
                            start=(b == 0),
                            stop=(b == SB - 1),
                            perf_mode=mybir.MatmulPerfMode.DoubleRow,
                        )
                        t += 1
                    if g == ng - 1:
                        # fold: out = psum_hi + psum_mid (no scaling).  ACT
                        # stages the mid half through SBUF (DVE can't read
                        # two PSUM operands in one op).
                        tm = opool.tile([WN, ng, D], f32, tag="t")
                        nc.scalar.copy(out=tm[:, :, :], in_=ps[:, :, D : 2 * D])
                        ot = opool.tile([WN, ng, D], f32, tag="o")
                        nc.vector.tensor_tensor(
                            out=ot[:, :, :],
                            in0=ps[:, :, 0:D],
                            in1=tm[:, :, :],
                            op=mybir.AluOpType.add,
                        )
                        # SWDGE ring: parallel to the HW-DGE load path
                        nc.gpsimd.dma_start(
                            out=odev[:, g0 * D : (w + 1) * D].rearrange(
                                "n (g f) -> n g f", g=ng
                            ),
                            in_=ot[:, :, :],
                        )
                        gi += 1
                        g0 = w + 1
    nc.compile()
    return nc, pkt, mskd, odev


# --------------------------------------------------------------------- main
def kernel(H, X_node, node_num):
    from concourse import bass_utils

    H32 = np.asarray(H, dtype=np.float32)
    X = np.asarray(X_node).astype(np.int64)
    N = int(node_num)
    E = X.shape[0]
    assert H32.shape == (E, D)

    sched, wins_core, cum, order = _plan(X, N)
    W = len(sched)

    blob, class_off = _build_masks(sched)
    MW = blob.shape[1]
    nc, pkt, mskd, odev = _build_program(sched, class_off, MW)
    in_maps = []
    for cr in range(N_CORES):
        pkt_np = _build_core_inputs(H32, cum, order, wins_core[cr], sched)
        in_maps.append({pkt.name: pkt_np, mskd.name: blob})

    trace = bool(int(os.environ.get("SEGSUM_TRACE", "0")))
    res = bass_utils.run_bass_kernel_spmd(
        nc, in_maps, core_ids=list(range(N_CORES)), trace=trace
    )
    if trace:
        kernel.last_exec_time_ns = res.exec_time_ns
        kernel.last_mean_exec_time_ns = res.mean_exec_time_ns
        kernel.last_trace = (
            res.instructions_and_trace[1] if res.instructions_and_trace else None
        )

    out = np.zeros((N, D), dtype=np.float32)
    for cr in range(N_CORES):
        ot = res.results[cr][odev.name].reshape(WN, W, D)  # [node_off, w, D]
        for w, nodes in enumerate(wins_core[cr]):
            if nodes:
                out[np.asarray(nodes), :] = ot[: len(nodes), w, :]
    return out
